# revision 15
# baseline (speedup 1.0000x reference)
"""v5: deep-banked global-chunk pipeline (g = r*NCH + c).

v4 critical path was f3(g-2) -> st issue -> ld(g) -> premask(g) -> Pool(g):
Pool start was chained to DVE fold completion through the input load. v5
quad-buffers raw/mk/idm, issues loads 3 chunks ahead, and premasks g+2
between f1(g) and f2(g), so Pool runs back-to-back:

  DVE:  pm(0) pm(1) | f1(0) pm(2) f2(0) f3(0)x4 | f1(1) pm(3) f2(1) ... |
  Pool: m2(0) m3(0) m2(1) m3(1) ...   (continuous)
  sync: ld(0..2) | ld(3) st(0) | ld(4) st(1) | ...
"""
import sys

for _p in ("/opt/trn_rl_repo", "/root/.axon_site/_ro/trn_rl_repo"):
    if _p not in sys.path:
        sys.path.append(_p)

import numpy as np

B, S, O, H = 64, 8192, 4, 16
NCORES = 8
BPC = B // NCORES
N = BPC * S
P = 128
KTOT = N // P                  # 512
KC = 256
NCH = KTOT // KC               # 2
NQ = 1
KQ = KC // NQ
TABLE = 1 << 20
MASK20 = TABLE - 1

_cache = {}


def _build(p1, p2, p3, iters=1):
    import concourse.bass as bass
    from concourse import mybir

    A = mybir.AluOpType
    I32 = mybir.dt.int32
    U8 = mybir.dt.uint8

    nc = bass.Bass()

    ids_d = nc.declare_dram_parameter("ids", [P, 4, KTOT], I32, isOutput=False)
    cst_d = nc.declare_dram_parameter("cst", [P, 3 * H], I32, isOutput=False)
    out_d = nc.declare_dram_parameter("out", [P, H, KTOT], I32, isOutput=True)

    NBUF = 4
    idp = [nc.alloc_sbuf_tensor(f"idp{b}", [P, 4, KC], I32) for b in range(NBUF)]
    cst = nc.alloc_sbuf_tensor("cst_t", [P, 3 * H], I32)
    mA = nc.alloc_sbuf_tensor("mA", [P, 1], I32)
    m1b = [nc.alloc_sbuf_tensor(f"m1b{c}", [P, H, KC], I32) for c in range(NCH)]
    m2b = [nc.alloc_sbuf_tensor(f"m2b{c}", [P, H, KC], I32) for c in range(NCH)]
    m3b = [nc.alloc_sbuf_tensor(f"m3b{c}", [P, H, KC], I32) for c in range(NCH)]
    f1b = nc.alloc_sbuf_tensor("f1b", [P, H, KC], I32)
    ot = [nc.alloc_sbuf_tensor(f"ot{c}", [P, H, KC], I32) for c in range(NCH)]

    s_in = nc.alloc_semaphore("s_in")      # +16 per chunk load
    s_m1 = nc.alloc_semaphore("s_m1")
    s_m2 = nc.alloc_semaphore("s_m2")
    s_m3 = nc.alloc_semaphore("s_m3")
    s_f1 = nc.alloc_semaphore("s_f1")
    s_f2 = nc.alloc_semaphore("s_f2")
    s_f = nc.alloc_semaphore("s_f")        # +1 per q-quarter of f3
    s_out = nc.alloc_semaphore("s_out")    # +16 per store

    G = NCH * iters

    with nc.Block() as block:
        @block.sync
        def _(sync: bass.BassEngine):
            sync.dma_start(out=cst[:], in_=cst_d[:]).then_inc(s_in, 16)

            def load(g):
                b, c = g % NBUF, g % NCH
                if g >= NBUF:
                    sync.wait_ge(s_m1, g - NBUF + 1)
                    sync.wait_ge(s_f, g - NBUF + 1)
                sync.dma_start(out=idp[b][:], in_=ids_d[:, :, c * KC:(c + 1) * KC]).then_inc(s_in, 16)

            def store(g):
                cp = g % NCH
                sync.wait_ge(s_f, g + 1)
                sync.dma_start(
                    out=out_d[:, :, cp * KC:(cp + 1) * KC],
                    in_=ot[g % 2][:],
                ).then_inc(s_out, 16)

            for g in range(min(3, G)):
                load(g)
            for g in range(G):
                if g + 3 < G:
                    load(g + 3)
                store(g)
            sync.wait_ge(s_out, 16 * G)

        @block.vector
        def _(v: bass.BassEngine):
            v.memset(mA[:], MASK20)
            for g in range(G):
                c = g % NCH
                b = g % NBUF
                v.wait_ge(s_m2, g + 1)
                v.wait_ge(s_m3, g + 1)
                v.tensor_tensor(f1b[:].rearrange("p h k -> p (h k)"),
                                m3b[c][:].rearrange("p h k -> p (h k)"),
                                m2b[c][:].rearrange("p h k -> p (h k)"),
                                A.bitwise_xor).then_inc(s_f1, 1)
                v.wait_ge(s_m1, g + 1)
                v.tensor_tensor(m3b[c][:].rearrange("p h k -> p (h k)"),
                                f1b[:].rearrange("p h k -> p (h k)"),
                                m1b[c][:].rearrange("p h k -> p (h k)"),
                                A.bitwise_xor).then_inc(s_f2, 1)
                if g >= 2:
                    v.wait_ge(s_out, 16 * (g - 1))
                id0q = idp[b][:, 0, :].rearrange("p (x k) -> p x k", x=1).broadcast_to([P, H, KC])
                v.scalar_tensor_tensor(ot[c][:], m3b[c][:], mA[:],
                                       id0q, A.bitwise_and, A.bitwise_xor).then_inc(s_f, 1)

        @block.scalar
        def _(sc: bass.BassEngine):
            for g in range(G):
                c = g % NCH
                b = g % NBUF
                sc.wait_ge(s_in, 16 + 16 * (g + 1))
                if g >= 2:
                    sc.wait_ge(s_f2, g - 1)
                for h in range(H):
                    ins = sc.mul(m1b[c][:, h, :], idp[b][:, 1, :], float(p1[h]))
                    if h == H - 1:
                        ins.then_inc(s_m1, 1)

        @block.gpsimd
        def _(gp: bass.BassEngine):
            for g in range(G):
                c = g % NCH
                b = g % NBUF
                HS = 2  # h-groups per stage: 4 concurrent ops fill the Q7 queue
                HG = H // HS
                gp.wait_ge(s_in, 16 + 16 * (g + 1))
                if g >= 2:
                    gp.wait_ge(s_f1, g - 1)
                for j in range(HS):
                    i2b = idp[b][:, 2, :].rearrange("p (x k) -> p x k", x=1).broadcast_to([P, HG, KC])
                    c2b = cst[:, j * HG:(j + 1) * HG].rearrange("p (h x) -> p h x", x=1).broadcast_to([P, HG, KC])
                    ins = gp.tensor_tensor(m2b[c][:, j * HG:(j + 1) * HG, :], i2b, c2b, A.mult)
                    if j == HS - 1:
                        ins.then_inc(s_m2, 1)
                if g >= 2:
                    gp.wait_ge(s_f, g - 1)
                for j in range(HS):
                    i3b = idp[b][:, 3, :].rearrange("p (x k) -> p x k", x=1).broadcast_to([P, HG, KC])
                    c3b = cst[:, H + j * HG:H + (j + 1) * HG].rearrange("p (h x) -> p h x", x=1).broadcast_to([P, HG, KC])
                    ins = gp.tensor_tensor(m3b[c][:, j * HG:(j + 1) * HG, :], i3b, c3b, A.mult)
                    if j == HS - 1:
                        ins.then_inc(s_m3, 1)

    return nc


def _prep(ngram_ids, ngram_mask, prime_powers):
    """Shared host-side prep: per-core input maps + prime constants."""
    ids = np.asarray(ngram_ids)
    msk = np.asarray(ngram_mask)
    pw = np.asarray(prime_powers)

    p1 = [int(x) for x in pw[:H, 1]]
    p2 = [int(x) for x in pw[:H, 2]]
    p3 = [int(x & 0xFFFFFFFF) for x in pw[:H, 3]]

    ids32 = ids.view(np.int32).reshape(B, S, 2 * O)[:, :, 0::2]   # low words
    ids32 = ids32 * msk[:, :, None].astype(np.int32)              # mask folded in

    cstv = np.empty((P, 3 * H), np.int32)
    cstv[:, :H] = np.asarray(p2, np.int64).astype(np.int32)[None, :]
    cstv[:, H:2 * H] = np.asarray(p3, np.uint32).view(np.int32)[None, :]
    cstv[:, 2 * H:] = np.asarray(p1, np.int64).astype(np.int32)[None, :]

    in_maps = []
    for c in range(NCORES):
        core_ids = np.ascontiguousarray(
            ids32[c * BPC:(c + 1) * BPC].reshape(P, KTOT, 4).transpose(0, 2, 1))
        in_maps.append({"ids": core_ids, "cst": cstv})
    return in_maps, p1, p2, p3


def kernel(ngram_ids, ngram_mask, prime_powers, table_size):
    from concourse.bass_utils import run_bass_kernel_spmd

    assert int(table_size) == TABLE
    ids = np.asarray(ngram_ids)
    pw = np.asarray(prime_powers)
    assert ids.shape == (B, S, O) and ids.dtype == np.int64
    assert pw.shape[1] >= 4 and np.all(pw[:, 0] == 1)

    in_maps, p1, p2, p3 = _prep(ngram_ids, ngram_mask, prime_powers)

    key = (tuple(p1), tuple(p2), tuple(p3))
    if key not in _cache:
        _cache[key] = _build(p1, p2, p3)
    nc = _cache[key]

    res = run_bass_kernel_spmd(nc, in_maps, list(range(NCORES)))

    out = np.empty((B, S, H), np.int64)
    for c in range(NCORES):
        o32 = res.results[c]["out"]
        out[c * BPC:(c + 1) * BPC] = o32.transpose(0, 2, 1).reshape(BPC, S, H).astype(np.int64)
    return out


if __name__ == "__main__":
    rng = np.random.default_rng(0)
    ids = rng.integers(0, 32000, size=(B, S, O)).astype(np.int64)
    msk = np.ones((B, S), dtype=bool)
    msk[3, 100:200] = False
    primes = np.array([31, 37, 41, 43, 47, 53, 59, 61, 67, 71, 73, 79, 83, 89, 97, 101], np.int64)
    pw = primes[:, None] ** np.arange(8, dtype=np.int64)[None, :]
    got = kernel(ids, msk, pw, TABLE)
    w = ids[:, :, :, None].astype(np.int64) * pw.T[:4][None, None, :, :]
    exp = w[..., 0, :]
    for i in range(1, 4):
        exp = exp ^ w[..., i, :]
    exp = (exp % TABLE) * msk[..., None]
    print("match:", np.array_equal(got, exp))
    bad = got != exp
    if bad.any():
        idx = np.argwhere(bad)
        print("nbad:", len(idx))
        for b_, s_, h_ in idx[:5]:
            print(b_, s_, h_, got[b_, s_, h_], exp[b_, s_, h_])


# revision 17
# speedup vs baseline: 1.0585x; 1.0585x over previous
"""v5: deep-banked global-chunk pipeline (g = r*NCH + c).

v4 critical path was f3(g-2) -> st issue -> ld(g) -> premask(g) -> Pool(g):
Pool start was chained to DVE fold completion through the input load. v5
quad-buffers raw/mk/idm, issues loads 3 chunks ahead, and premasks g+2
between f1(g) and f2(g), so Pool runs back-to-back:

  DVE:  pm(0) pm(1) | f1(0) pm(2) f2(0) f3(0)x4 | f1(1) pm(3) f2(1) ... |
  Pool: m2(0) m3(0) m2(1) m3(1) ...   (continuous)
  sync: ld(0..2) | ld(3) st(0) | ld(4) st(1) | ...
"""
import sys

for _p in ("/opt/trn_rl_repo", "/root/.axon_site/_ro/trn_rl_repo"):
    if _p not in sys.path:
        sys.path.append(_p)

import numpy as np

B, S, O, H = 64, 8192, 4, 16
NCORES = 8
BPC = B // NCORES
N = BPC * S
P = 128
KTOT = N // P                  # 512
KC = 256
NCH = KTOT // KC               # 2
NQ = 1
KQ = KC // NQ
TABLE = 1 << 20
MASK20 = TABLE - 1

_cache = {}


def _build(p1, p2, p3, iters=1):
    import concourse.bass as bass
    from concourse import mybir

    A = mybir.AluOpType
    I32 = mybir.dt.int32
    U8 = mybir.dt.uint8

    nc = bass.Bass()

    ids_d = nc.declare_dram_parameter("ids", [P, 4, KTOT], I32, isOutput=False)
    cst_d = nc.declare_dram_parameter("cst", [P, 3 * H], I32, isOutput=False)
    out_d = nc.declare_dram_parameter("out", [P, H, KTOT], I32, isOutput=True)

    NBUF = 4
    idp = [nc.alloc_sbuf_tensor(f"idp{b}", [P, 4, KC], I32) for b in range(NBUF)]
    cst = nc.alloc_sbuf_tensor("cst_t", [P, 3 * H], I32)
    mA = nc.alloc_sbuf_tensor("mA", [P, 1], I32)
    m1b = [nc.alloc_sbuf_tensor(f"m1b{c}", [P, H, KC], I32) for c in range(NCH)]
    m2b = [nc.alloc_sbuf_tensor(f"m2b{c}", [P, H, KC], I32) for c in range(NCH)]
    m3b = [nc.alloc_sbuf_tensor(f"m3b{c}", [P, H, KC], I32) for c in range(NCH)]
    f1b = nc.alloc_sbuf_tensor("f1b", [P, H, KC], I32)
    ot = [nc.alloc_sbuf_tensor(f"ot{c}", [P, H, KC], I32) for c in range(NCH)]

    s_in = nc.alloc_semaphore("s_in")      # +16 per chunk load
    s_m1 = nc.alloc_semaphore("s_m1")
    s_m2 = nc.alloc_semaphore("s_m2")
    s_m3 = nc.alloc_semaphore("s_m3")
    s_f1 = nc.alloc_semaphore("s_f1")
    s_f2 = nc.alloc_semaphore("s_f2")
    s_f = nc.alloc_semaphore("s_f")        # +1 per q-quarter of f3
    s_out = nc.alloc_semaphore("s_out")    # +16 per store

    G = NCH * iters

    with nc.Block() as block:
        @block.sync
        def _(sync: bass.BassEngine):
            sync.dma_start(out=cst[:], in_=cst_d[:]).then_inc(s_in, 16)

            def load(g):
                b, c = g % NBUF, g % NCH
                if g >= NBUF:
                    sync.wait_ge(s_m1, g - NBUF + 1)
                    sync.wait_ge(s_f, g - NBUF + 1)
                sync.dma_start(out=idp[b][:], in_=ids_d[:, :, c * KC:(c + 1) * KC]).then_inc(s_in, 16)

            def store(g):
                cp = g % NCH
                sync.wait_ge(s_f, g + 1)
                sync.dma_start(
                    out=out_d[:, :, cp * KC:(cp + 1) * KC],
                    in_=ot[g % 2][:],
                ).then_inc(s_out, 16)

            for g in range(min(3, G)):
                load(g)
            for g in range(G):
                if g + 3 < G:
                    load(g + 3)
                store(g)
            sync.wait_ge(s_out, 16 * G)

        @block.vector
        def _(v: bass.BassEngine):
            v.memset(mA[:], MASK20)
            for g in range(G):
                c = g % NCH
                b = g % NBUF
                v.wait_ge(s_m2, g + 1)
                v.wait_ge(s_m3, g + 1)
                v.tensor_tensor(f1b[:].rearrange("p h k -> p (h k)"),
                                m3b[c][:].rearrange("p h k -> p (h k)"),
                                m2b[c][:].rearrange("p h k -> p (h k)"),
                                A.bitwise_xor).then_inc(s_f1, 1)
                v.wait_ge(s_m1, g + 1)
                v.tensor_tensor(m3b[c][:].rearrange("p h k -> p (h k)"),
                                f1b[:].rearrange("p h k -> p (h k)"),
                                m1b[c][:].rearrange("p h k -> p (h k)"),
                                A.bitwise_xor).then_inc(s_f2, 1)
                if g >= 2:
                    v.wait_ge(s_out, 16 * (g - 1))
                id0q = idp[b][:, 0, :].rearrange("p (x k) -> p x k", x=1).broadcast_to([P, H, KC])
                v.scalar_tensor_tensor(ot[c][:], m3b[c][:], mA[:],
                                       id0q, A.bitwise_and, A.bitwise_xor).then_inc(s_f, 1)

        @block.scalar
        def _(sc: bass.BassEngine):
            for g in range(G):
                c = g % NCH
                b = g % NBUF
                sc.wait_ge(s_in, 16 + 16 * (g + 1))
                if g >= 2:
                    sc.wait_ge(s_f2, g - 1)
                for h in range(H):
                    ins = sc.mul(m1b[c][:, h, :], idp[b][:, 1, :], float(p1[h]))
                    if h == H - 1:
                        ins.then_inc(s_m1, 1)

        @block.gpsimd
        def _(gp: bass.BassEngine):
            for g in range(G):
                c = g % NCH
                b = g % NBUF
                HS = 8  # h-groups per stage: 4 concurrent ops fill the Q7 queue
                HG = H // HS
                gp.wait_ge(s_in, 16 + 16 * (g + 1))
                if g >= 2:
                    gp.wait_ge(s_f1, g - 1)
                for j in range(HS):
                    i2b = idp[b][:, 2, :].rearrange("p (x k) -> p x k", x=1).broadcast_to([P, HG, KC])
                    c2b = cst[:, j * HG:(j + 1) * HG].rearrange("p (h x) -> p h x", x=1).broadcast_to([P, HG, KC])
                    ins = gp.tensor_tensor(m2b[c][:, j * HG:(j + 1) * HG, :], i2b, c2b, A.mult)
                    if j == HS - 1:
                        ins.then_inc(s_m2, 1)
                if g >= 2:
                    gp.wait_ge(s_f, g - 1)
                for j in range(HS):
                    i3b = idp[b][:, 3, :].rearrange("p (x k) -> p x k", x=1).broadcast_to([P, HG, KC])
                    c3b = cst[:, H + j * HG:H + (j + 1) * HG].rearrange("p (h x) -> p h x", x=1).broadcast_to([P, HG, KC])
                    ins = gp.tensor_tensor(m3b[c][:, j * HG:(j + 1) * HG, :], i3b, c3b, A.mult)
                    if j == HS - 1:
                        ins.then_inc(s_m3, 1)

    return nc


def _prep(ngram_ids, ngram_mask, prime_powers):
    """Shared host-side prep: per-core input maps + prime constants."""
    ids = np.asarray(ngram_ids)
    msk = np.asarray(ngram_mask)
    pw = np.asarray(prime_powers)

    p1 = [int(x) for x in pw[:H, 1]]
    p2 = [int(x) for x in pw[:H, 2]]
    p3 = [int(x & 0xFFFFFFFF) for x in pw[:H, 3]]

    ids32 = ids.view(np.int32).reshape(B, S, 2 * O)[:, :, 0::2]   # low words
    ids32 = ids32 * msk[:, :, None].astype(np.int32)              # mask folded in

    cstv = np.empty((P, 3 * H), np.int32)
    cstv[:, :H] = np.asarray(p2, np.int64).astype(np.int32)[None, :]
    cstv[:, H:2 * H] = np.asarray(p3, np.uint32).view(np.int32)[None, :]
    cstv[:, 2 * H:] = np.asarray(p1, np.int64).astype(np.int32)[None, :]

    in_maps = []
    for c in range(NCORES):
        core_ids = np.ascontiguousarray(
            ids32[c * BPC:(c + 1) * BPC].reshape(P, KTOT, 4).transpose(0, 2, 1))
        in_maps.append({"ids": core_ids, "cst": cstv})
    return in_maps, p1, p2, p3


def kernel(ngram_ids, ngram_mask, prime_powers, table_size):
    from concourse.bass_utils import run_bass_kernel_spmd

    assert int(table_size) == TABLE
    ids = np.asarray(ngram_ids)
    pw = np.asarray(prime_powers)
    assert ids.shape == (B, S, O) and ids.dtype == np.int64
    assert pw.shape[1] >= 4 and np.all(pw[:, 0] == 1)

    in_maps, p1, p2, p3 = _prep(ngram_ids, ngram_mask, prime_powers)

    key = (tuple(p1), tuple(p2), tuple(p3))
    if key not in _cache:
        _cache[key] = _build(p1, p2, p3)
    nc = _cache[key]

    res = run_bass_kernel_spmd(nc, in_maps, list(range(NCORES)))

    out = np.empty((B, S, H), np.int64)
    for c in range(NCORES):
        o32 = res.results[c]["out"]
        out[c * BPC:(c + 1) * BPC] = o32.transpose(0, 2, 1).reshape(BPC, S, H).astype(np.int64)
    return out


if __name__ == "__main__":
    rng = np.random.default_rng(0)
    ids = rng.integers(0, 32000, size=(B, S, O)).astype(np.int64)
    msk = np.ones((B, S), dtype=bool)
    msk[3, 100:200] = False
    primes = np.array([31, 37, 41, 43, 47, 53, 59, 61, 67, 71, 73, 79, 83, 89, 97, 101], np.int64)
    pw = primes[:, None] ** np.arange(8, dtype=np.int64)[None, :]
    got = kernel(ids, msk, pw, TABLE)
    w = ids[:, :, :, None].astype(np.int64) * pw.T[:4][None, None, :, :]
    exp = w[..., 0, :]
    for i in range(1, 4):
        exp = exp ^ w[..., i, :]
    exp = (exp % TABLE) * msk[..., None]
    print("match:", np.array_equal(got, exp))
    bad = got != exp
    if bad.any():
        idx = np.argwhere(bad)
        print("nbad:", len(idx))
        for b_, s_, h_ in idx[:5]:
            print(b_, s_, h_, got[b_, s_, h_], exp[b_, s_, h_])


# revision 18
# speedup vs baseline: 1.1341x; 1.0714x over previous
"""v5: deep-banked global-chunk pipeline (g = r*NCH + c).

v4 critical path was f3(g-2) -> st issue -> ld(g) -> premask(g) -> Pool(g):
Pool start was chained to DVE fold completion through the input load. v5
quad-buffers raw/mk/idm, issues loads 3 chunks ahead, and premasks g+2
between f1(g) and f2(g), so Pool runs back-to-back:

  DVE:  pm(0) pm(1) | f1(0) pm(2) f2(0) f3(0)x4 | f1(1) pm(3) f2(1) ... |
  Pool: m2(0) m3(0) m2(1) m3(1) ...   (continuous)
  sync: ld(0..2) | ld(3) st(0) | ld(4) st(1) | ...
"""
import sys

for _p in ("/opt/trn_rl_repo", "/root/.axon_site/_ro/trn_rl_repo"):
    if _p not in sys.path:
        sys.path.append(_p)

import numpy as np

B, S, O, H = 64, 8192, 4, 16
NCORES = 8
BPC = B // NCORES
N = BPC * S
P = 128
KTOT = N // P                  # 512
KC = 256
NCH = KTOT // KC               # 2
NQ = 1
KQ = KC // NQ
TABLE = 1 << 20
MASK20 = TABLE - 1

_cache = {}


def _build(p1, p2, p3, iters=1):
    import concourse.bass as bass
    from concourse import mybir

    A = mybir.AluOpType
    I32 = mybir.dt.int32
    U8 = mybir.dt.uint8

    nc = bass.Bass()

    ids_d = nc.declare_dram_parameter("ids", [P, 4, KTOT], I32, isOutput=False)
    cst_d = nc.declare_dram_parameter("cst", [P, 3 * H], I32, isOutput=False)
    out_d = nc.declare_dram_parameter("out", [P, H, KTOT], I32, isOutput=True)

    NBUF = 4
    idp = [nc.alloc_sbuf_tensor(f"idp{b}", [P, 4, KC], I32) for b in range(NBUF)]
    cst = nc.alloc_sbuf_tensor("cst_t", [P, 3 * H], I32)
    mA = nc.alloc_sbuf_tensor("mA", [P, 1], I32)
    m1b = [nc.alloc_sbuf_tensor(f"m1b{c}", [P, H, KC], I32) for c in range(NCH)]
    m2b = [nc.alloc_sbuf_tensor(f"m2b{c}", [P, H, KC], I32) for c in range(NCH)]
    m3b = [nc.alloc_sbuf_tensor(f"m3b{c}", [P, H, KC], I32) for c in range(NCH)]
    f1b = nc.alloc_sbuf_tensor("f1b", [P, H, KC], I32)
    ot = [nc.alloc_sbuf_tensor(f"ot{c}", [P, H, KC], I32) for c in range(NCH)]

    s_in = nc.alloc_semaphore("s_in")      # +16 per chunk load
    s_m1 = nc.alloc_semaphore("s_m1")
    s_m2 = nc.alloc_semaphore("s_m2")
    s_m3 = nc.alloc_semaphore("s_m3")
    s_f1 = nc.alloc_semaphore("s_f1")
    s_f2 = nc.alloc_semaphore("s_f2")
    s_f = nc.alloc_semaphore("s_f")        # +1 per q-quarter of f3
    s_out = nc.alloc_semaphore("s_out")    # +16 per store

    G = NCH * iters

    with nc.Block() as block:
        @block.sync
        def _(sync: bass.BassEngine):
            sync.dma_start(out=cst[:], in_=cst_d[:]).then_inc(s_in, 16)

            def load(g):
                b, c = g % NBUF, g % NCH
                if g >= NBUF:
                    sync.wait_ge(s_m1, g - NBUF + 1)
                    sync.wait_ge(s_f, g - NBUF + 1)
                sync.dma_start(out=idp[b][:], in_=ids_d[:, :, c * KC:(c + 1) * KC]).then_inc(s_in, 16)

            def store(g):
                cp = g % NCH
                sync.wait_ge(s_f, g + 1)
                sync.dma_start(
                    out=out_d[:, :, cp * KC:(cp + 1) * KC],
                    in_=ot[g % 2][:],
                ).then_inc(s_out, 16)

            for g in range(min(3, G)):
                load(g)
            for g in range(G):
                if g + 3 < G:
                    load(g + 3)
                store(g)
            sync.wait_ge(s_out, 16 * G)

        @block.vector
        def _(v: bass.BassEngine):
            v.memset(mA[:], MASK20)
            for g in range(G):
                c = g % NCH
                b = g % NBUF
                v.wait_ge(s_m2, g + 1)
                v.wait_ge(s_m3, g + 1)
                v.tensor_tensor(f1b[:].rearrange("p h k -> p (h k)"),
                                m3b[c][:].rearrange("p h k -> p (h k)"),
                                m2b[c][:].rearrange("p h k -> p (h k)"),
                                A.bitwise_xor).then_inc(s_f1, 1)
                v.wait_ge(s_m1, g + 1)
                v.tensor_tensor(m3b[c][:].rearrange("p h k -> p (h k)"),
                                f1b[:].rearrange("p h k -> p (h k)"),
                                m1b[c][:].rearrange("p h k -> p (h k)"),
                                A.bitwise_xor).then_inc(s_f2, 1)
                if g >= 2:
                    v.wait_ge(s_out, 16 * (g - 1))
                id0q = idp[b][:, 0, :].rearrange("p (x k) -> p x k", x=1).broadcast_to([P, H, KC])
                v.scalar_tensor_tensor(ot[c][:], m3b[c][:], mA[:],
                                       id0q, A.bitwise_and, A.bitwise_xor).then_inc(s_f, 1)

        @block.scalar
        def _(sc: bass.BassEngine):
            for g in range(G):
                c = g % NCH
                b = g % NBUF
                sc.wait_ge(s_in, 16 + 16 * (g + 1))
                if g >= 2:
                    sc.wait_ge(s_f2, g - 1)
                for h in range(H):
                    ins = sc.mul(m1b[c][:, h, :], idp[b][:, 1, :], float(p1[h]))
                    if h == H - 1:
                        ins.then_inc(s_m1, 1)

        @block.gpsimd
        def _(gp: bass.BassEngine):
            for g in range(G):
                c = g % NCH
                b = g % NBUF
                HS = 4  # h-groups per stage: 4 concurrent ops fill the Q7 queue
                HG = H // HS
                gp.wait_ge(s_in, 16 + 16 * (g + 1))
                if g >= 2:
                    gp.wait_ge(s_f1, g - 1)
                for j in range(HS):
                    i2b = idp[b][:, 2, :].rearrange("p (x k) -> p x k", x=1).broadcast_to([P, HG, KC])
                    c2b = cst[:, j * HG:(j + 1) * HG].rearrange("p (h x) -> p h x", x=1).broadcast_to([P, HG, KC])
                    ins = gp.tensor_tensor(m2b[c][:, j * HG:(j + 1) * HG, :], i2b, c2b, A.mult)
                    if j == HS - 1:
                        ins.then_inc(s_m2, 1)
                if g >= 2:
                    gp.wait_ge(s_f, g - 1)
                for j in range(HS):
                    i3b = idp[b][:, 3, :].rearrange("p (x k) -> p x k", x=1).broadcast_to([P, HG, KC])
                    c3b = cst[:, H + j * HG:H + (j + 1) * HG].rearrange("p (h x) -> p h x", x=1).broadcast_to([P, HG, KC])
                    ins = gp.tensor_tensor(m3b[c][:, j * HG:(j + 1) * HG, :], i3b, c3b, A.mult)
                    if j == HS - 1:
                        ins.then_inc(s_m3, 1)

    return nc


def _prep(ngram_ids, ngram_mask, prime_powers):
    """Shared host-side prep: per-core input maps + prime constants."""
    ids = np.asarray(ngram_ids)
    msk = np.asarray(ngram_mask)
    pw = np.asarray(prime_powers)

    p1 = [int(x) for x in pw[:H, 1]]
    p2 = [int(x) for x in pw[:H, 2]]
    p3 = [int(x & 0xFFFFFFFF) for x in pw[:H, 3]]

    ids32 = ids.view(np.int32).reshape(B, S, 2 * O)[:, :, 0::2]   # low words
    ids32 = ids32 * msk[:, :, None].astype(np.int32)              # mask folded in

    cstv = np.empty((P, 3 * H), np.int32)
    cstv[:, :H] = np.asarray(p2, np.int64).astype(np.int32)[None, :]
    cstv[:, H:2 * H] = np.asarray(p3, np.uint32).view(np.int32)[None, :]
    cstv[:, 2 * H:] = np.asarray(p1, np.int64).astype(np.int32)[None, :]

    in_maps = []
    for c in range(NCORES):
        core_ids = np.ascontiguousarray(
            ids32[c * BPC:(c + 1) * BPC].reshape(P, KTOT, 4).transpose(0, 2, 1))
        in_maps.append({"ids": core_ids, "cst": cstv})
    return in_maps, p1, p2, p3


def kernel(ngram_ids, ngram_mask, prime_powers, table_size):
    from concourse.bass_utils import run_bass_kernel_spmd

    assert int(table_size) == TABLE
    ids = np.asarray(ngram_ids)
    pw = np.asarray(prime_powers)
    assert ids.shape == (B, S, O) and ids.dtype == np.int64
    assert pw.shape[1] >= 4 and np.all(pw[:, 0] == 1)

    in_maps, p1, p2, p3 = _prep(ngram_ids, ngram_mask, prime_powers)

    key = (tuple(p1), tuple(p2), tuple(p3))
    if key not in _cache:
        _cache[key] = _build(p1, p2, p3)
    nc = _cache[key]

    res = run_bass_kernel_spmd(nc, in_maps, list(range(NCORES)))

    out = np.empty((B, S, H), np.int64)
    for c in range(NCORES):
        o32 = res.results[c]["out"]
        out[c * BPC:(c + 1) * BPC] = o32.transpose(0, 2, 1).reshape(BPC, S, H).astype(np.int64)
    return out


if __name__ == "__main__":
    rng = np.random.default_rng(0)
    ids = rng.integers(0, 32000, size=(B, S, O)).astype(np.int64)
    msk = np.ones((B, S), dtype=bool)
    msk[3, 100:200] = False
    primes = np.array([31, 37, 41, 43, 47, 53, 59, 61, 67, 71, 73, 79, 83, 89, 97, 101], np.int64)
    pw = primes[:, None] ** np.arange(8, dtype=np.int64)[None, :]
    got = kernel(ids, msk, pw, TABLE)
    w = ids[:, :, :, None].astype(np.int64) * pw.T[:4][None, None, :, :]
    exp = w[..., 0, :]
    for i in range(1, 4):
        exp = exp ^ w[..., i, :]
    exp = (exp % TABLE) * msk[..., None]
    print("match:", np.array_equal(got, exp))
    bad = got != exp
    if bad.any():
        idx = np.argwhere(bad)
        print("nbad:", len(idx))
        for b_, s_, h_ in idx[:5]:
            print(b_, s_, h_, got[b_, s_, h_], exp[b_, s_, h_])
